# revision 1
# baseline (speedup 1.0000x reference)
"""Trainium2 Bass kernel for nn_Attention_59030030516520.

Fused attention block: qkv projection + per-head RMSNorm + segmented RoPE +
softmax attention + output projection, distributed over 8 NeuronCores as
batch(2) x head-groups(4).  Each core computes 4 heads of one batch element
and a partial output projection; the host sums the partials and adds the bias.

Matmuls run in float32r (TF32-class, full PE rate); softmax exploits the bound
|q.k|/sqrt(D) <= sqrt(D) after RMSNorm so no max-subtraction pass is needed.
Scores are computed transposed (S^T = k q^T) so softmax rowsums come free via
a phantom all-ones v'-column and no transposes of the probability matrix are
required.
"""
import sys
sys.path.insert(0, "/opt/trn_rl_repo")
import numpy as np
import concourse.bass as bass
import concourse.mybir as mybir
import concourse.tile as tile
from concourse import bacc

F32 = mybir.dt.float32
F32R = mybir.dt.float32r
AF = mybir.ActivationFunctionType
ALU = mybir.AluOpType

B, N, C = 2, 2048, 1024
H, D = 16, 64
HPC = 4            # heads per core
NT = N // 128      # 16 seq tiles
QC = N // 512      # 4 q-chunks
EPS = 1e-6
SCALE = 1.0 / np.sqrt(D)
ROPE_SEGMENTS = (1024, 512)
NROPE = 1536
ROPE_THETA = 10000.0


def build_kernel(w_is_ones=True):
    nc = bacc.Bacc("TRN2", target_bir_lowering=False, debug=False)

    # ---- DRAM I/O (per-core) ----
    xT_d = nc.dram_tensor("xT", [C, N], F32R, kind="ExternalInput")           # x[b].T
    wqkT_d = nc.dram_tensor("wqkT", [C, 512], F32R, kind="ExternalInput")     # q,k weights.T (4 heads)
    bqk_d = nc.dram_tensor("bqk", [128, 4], F32, kind="ExternalInput")        # q,k bias per feature tile
    wvT_d = nc.dram_tensor("wvT", [C, 260], F32R, kind="ExternalInput")       # v weights.T + phantom cols
    bv_d = nc.dram_tensor("bv", [128, 260], F32, kind="ExternalInput")        # v bias row broadcast + ones at phantom
    cosF_d = nc.dram_tensor("cosF", [128, N], F32, kind="ExternalInput")
    sinF_d = nc.dram_tensor("sinF", [128, N], F32, kind="ExternalInput")
    wq_d = nc.dram_tensor("wq", [128, 1], F32, kind="ExternalInput")          # qn_w tiled
    wk_d = nc.dram_tensor("wk", [128, 1], F32, kind="ExternalInput")
    ind_d = nc.dram_tensor("ind", [128, 33], F32R, kind="ExternalInput")       # 64-row group indicator
    wpT_d = nc.dram_tensor("wpT", [256, C], F32R, kind="ExternalInput")       # proj weights slice.T
    yT_d = nc.dram_tensor("yT", [C, N], F32, kind="ExternalOutput")           # partial proj out.T

    with tile.TileContext(nc) as tc:
        with (
            tc.tile_pool(name="pers", bufs=1) as pers,     # persistent tensors (unique tags)
            tc.tile_pool(name="big", bufs=11) as big,      # recycled [128,2048] working tiles
            tc.tile_pool(name="vp", bufs=16) as vpool,     # v' tiles live through attention
            tc.tile_pool(name="sm", bufs=4) as sm,         # small working tiles
            tc.tile_pool(name="ps", bufs=2, space="PSUM") as psum_s,   # 2x2 banks
            tc.tile_pool(name="po", bufs=2, space="PSUM") as psum_o,   # 2 banks
            tc.tile_pool(name="pm", bufs=2, space="PSUM") as psum_m,   # 2 banks
        ):
            # ---- load weights/constants ----
            wqkT = [pers.tile([128, 512], F32R, tag=f"wqk{i}", name=f"wqk{i}") for i in range(8)]
            nc.sync.dma_start(wqkT[0][:], wqkT_d[0:128, :])
            nc.scalar.dma_start(wqkT[1][:], wqkT_d[128:256, :])
            xT = [big.tile([128, N], F32R, tag="big", name=f"xT{i}") for i in range(8)]
            _eng = [nc.sync, nc.scalar]
            for i in range(8):
                _eng[i % 2].dma_start(xT[i][:], xT_d[128 * i:128 * (i + 1), :])

            for i in range(2, 8):
                [nc.sync, nc.scalar][i % 2].dma_start(wqkT[i][:], wqkT_d[128 * i:128 * (i + 1), :])
            wvT = [pers.tile([128, 260], F32R, tag=f"wv{i}", name=f"wv{i}") for i in range(8)]
            for i in range(8):
                [nc.sync, nc.scalar][i % 2].dma_start(wvT[i][:], wvT_d[128 * i:128 * (i + 1), :])
            wpT = [pers.tile([128, C], F32R, tag=f"wp{i}", name=f"wp{i}") for i in range(2)]
            bqk = pers.tile([128, 4], F32, tag="bqk")
            nc.sync.dma_start(bqk[:], bqk_d[:])
            bv = pers.tile([128, 260], F32, tag="bv")
            nc.scalar.dma_start(bv[:], bv_d[:])
            wq = pers.tile([128, 1], F32, tag="wq")
            nc.sync.dma_start(wq[:], wq_d[:])
            wk = pers.tile([128, 1], F32, tag="wk")
            nc.scalar.dma_start(wk[:], wk_d[:])
            ind = pers.tile([128, 33], F32R, tag="ind")
            nc.sync.dma_start(ind[:], ind_d[:])
            eps_t = pers.tile([64, 1], F32, tag="eps", name="eps_t")
            nc.vector.memset(eps_t[:], EPS)

            # ---- qkv: q,k channel-major [feature, seq] ----
            # qkf[0],qkf[1] = q heads (0,1),(2,3); qkf[2],qkf[3] = k heads
            # fp32 scratch lives in the persistent f32r tiles (bitcast views)
            qkf = [pers.tile([128, N], F32R, tag=f"qkf{t}", name=f"qkf{t}") for t in range(4)]
            raw = [big.tile([128, N], F32, tag="big", name=f"raw{t}") for t in range(4)]
            for ft in (0, 2, 1, 3):
                for half in range(2):
                    ps = psum_s.tile([128, 1024], F32, tag="s")
                    for ci in range(8):
                        for q2 in range(2):
                            qc = 2 * half + q2
                            nc.tensor.matmul(
                                ps[:, 512 * q2:512 * (q2 + 1)],
                                wqkT[ci][:, 128 * ft:128 * (ft + 1)],
                                xT[ci][:, 512 * qc:512 * (qc + 1)],
                                start=(ci == 0), stop=(ci == 7),
                            )
                    # r = psum + bias (per-partition)
                    nc.vector.tensor_scalar(raw[ft][:, 1024 * half:1024 * (half + 1)],
                                            ps[:], bqk[:, ft:ft + 1], None, ALU.add)

            # ---- v: seq-major [seq, 65*4] with phantom ones columns ----
            vp = []
            for st in range(NT):
                ps = psum_m.tile([128, 512], F32, tag="m")
                for ci in range(8):
                    nc.tensor.matmul(
                        ps[:, :260],
                        xT[ci][:, 128 * st:128 * (st + 1)],
                        wvT[ci][:],
                        start=(ci == 0), stop=(ci == 7),
                    )
                v = vpool.tile([128, 260], F32R, tag="v")
                nc.vector.tensor_tensor(v[:], ps[:, :260], bv[:], ALU.add)
                vp.append(v)

            aT = [pers.tile([128, N], F32R, tag=f"aT{i}", name=f"aT{i}") for i in range(2)]
            cosF = big.tile([128, N], F32, tag="big", name="cosF")
            nc.scalar.dma_start(cosF[:], cosF_d[:])
            sinF = big.tile([128, N], F32, tag="big", name="sinF")
            nc.scalar.dma_start(sinF[:], sinF_d[:])

            # ---- per-tile: RMSNorm stats + RoPE + ir scaling (t order 0,2 first
            # so attention heads 0/1 can start while tiles 1/3 normalize) ----
            def norm_tile(t):
                sq = big.tile([128, N], F32R, tag="big")
                ir = sm.tile([64, N], F32, tag="ir", bufs=1, name=f"ir{t}")
                for qc in range(QC):
                    nc.vector.tensor_tensor(sq[:, 512 * qc:512 * (qc + 1)],
                                            raw[t][:, 512 * qc:512 * (qc + 1)],
                                            raw[t][:, 512 * qc:512 * (qc + 1)], ALU.mult)
                    pr = psum_m.tile([128, 512], F32, tag="m")
                    sl = sq[:, 512 * qc:512 * (qc + 1)]
                    nc.tensor.matmul(pr[0:33, :512], ind[:], sl, start=True, stop=True)
                    # ir = 1/sqrt(ssq/D + eps); all norm Sqrts precede all
                    # softmax Exps (order A), so only 2 ACT table loads total
                    nc.scalar.activation(
                        ir[0:33, 512 * qc:512 * (qc + 1)],
                        pr[0:33, :512], AF.Sqrt,
                        bias=eps_t[0:33], scale=1.0 / D,
                    )
                    nc.vector.reciprocal(ir[0:33, 512 * qc:512 * (qc + 1)],
                                         ir[0:33, 512 * qc:512 * (qc + 1)])
                if not w_is_ones:
                    # exact general w: scale channels before rope (after stats)
                    wvec = wq if t < 2 else wk
                    nc.vector.tensor_scalar(raw[t][:], raw[t][:], wvec[:], None, ALU.mult)
                sw = big.tile([128, NROPE], F32, tag="big")
                for blk in range(4):
                    sfrom = (blk // 2) * 64 + (32 if blk % 2 == 0 else 0)
                    sto = (blk // 2) * 64 + (0 if blk % 2 == 0 else 32)
                    [nc.scalar, nc.sync][blk % 2].dma_start(sw[sto:sto + 32, :], raw[t][sfrom:sfrom + 32, 0:NROPE])
                # rope out-of-place so stats and rope chains overlap
                ropeo = big.tile([128, NROPE], F32, tag="big")
                nc.vector.tensor_tensor(ropeo[:], raw[t][:, 0:NROPE], cosF[:, 0:NROPE], ALU.mult)
                nc.vector.tensor_tensor(sw[:], sw[:], sinF[:, 0:NROPE], ALU.mult)
                nc.vector.tensor_tensor(ropeo[:], ropeo[:], sw[:], ALU.add)
                # broadcast ir rows to 64-row blocks (partition_broadcast only
                # works base0->base0 on HW; shift the second half with DMA)
                bc = big.tile([128, N], F32, tag="big")
                tmpb = big.tile([64, N], F32, tag="big")
                nc.gpsimd.dma_start(tmpb[0:1, :], ir[32:33, :])
                nc.gpsimd.partition_broadcast(bc[0:64, :], ir[0:1, :])
                nc.gpsimd.partition_broadcast(tmpb[0:64, :], tmpb[0:1, :])
                nc.sync.dma_start(bc[64:128, :], tmpb[0:64, :])
                nc.vector.tensor_tensor(qkf[t][:, 0:NROPE], bc[:, 0:NROPE], ropeo[:], ALU.mult)
                nc.vector.tensor_tensor(qkf[t][:, NROPE:N], bc[:, NROPE:N], raw[t][:, NROPE:N], ALU.mult)


            # ---- attention chain for one (qc, head) ----
            def attn_chain(qc, hl):
                ti, ro = hl // 2, 64 * (hl % 2)
                qf, kf = qkf[ti], qkf[2 + ti]
                po = psum_o.tile([128, 512], F32, tag="o", name=f"po{qc}_{hl}")
                for grp in range(8):
                    s2 = psum_s.tile([128, 1024], F32, tag="s", name=f"s{qc}_{hl}_{grp}")
                    for b2 in range(2):
                        t = 2 * grp + b2
                        nc.tensor.matmul(
                            s2[:, 512 * b2:512 * (b2 + 1)],
                            kf[ro:ro + 64, 128 * t:128 * (t + 1)],
                            qf[ro:ro + 64, 512 * qc:512 * (qc + 1)],
                            start=True, stop=True,
                        )
                    p2 = big.tile([128, 1024], F32R, tag="big", name=f"p{qc}_{hl}_{grp}")
                    nc.scalar.activation(p2[:], s2[:], AF.Exp, scale=float(SCALE))
                    for b2 in range(2):
                        t = 2 * grp + b2
                        nc.tensor.matmul(
                            po[0:65, :512],
                            vp[t][:, 65 * hl:65 * (hl + 1)],
                            p2[:, 512 * b2:512 * (b2 + 1)],
                            start=(t == 0), stop=(t == 15),
                        )
                # normalize: recip of rowsum (row 64), broadcast, multiply
                rs = sm.tile([128, 512], F32, tag="rs", bufs=1, name=f"rs{qc}_{hl}")
                nc.vector.reciprocal(rs[64:65, :], po[64:65, :512])
                nc.gpsimd.dma_start(rs[0:1, :], rs[64:65, :])
                rbc = sm.tile([64, 512], F32, tag="rbc", bufs=1, name=f"rbc{qc}_{hl}")
                nc.gpsimd.partition_broadcast(rbc[:], rs[0:1, :])
                if hl % 2 == 0:
                    nc.vector.tensor_tensor(
                        aT[ti][0:64, 512 * qc:512 * (qc + 1)],
                        po[0:64, :512], rbc[:], ALU.mult)
                else:
                    tmp = sm.tile([64, 512], F32R, tag="tmp", bufs=1, name=f"tmp{qc}_{hl}")
                    nc.vector.tensor_tensor(tmp[:], po[0:64, :512], rbc[:], ALU.mult)
                    nc.scalar.dma_start(aT[ti][64:128, 512 * qc:512 * (qc + 1)], tmp[:])

            def proj_qc(qc):
                for ot in range(8):
                    yp = psum_m.tile([128, 512], F32, tag="m", name=f"yp{qc}_{ot}")
                    for c2 in range(2):
                        nc.tensor.matmul(
                            yp[:, :512],
                            wpT[c2][:, 128 * ot:128 * (ot + 1)],
                            aT[c2][:, 512 * qc:512 * (qc + 1)],
                            start=(c2 == 0), stop=(c2 == 1),
                        )
                    yo = sm.tile([128, 512], F32, tag="yo", name=f"yo{qc}_{ot}", bufs=2)
                    if qc == QC - 1 and ot % 2 == 1:
                        nc.scalar.copy(yo[:], yp[:, :512])
                    else:
                        nc.vector.tensor_copy(yo[:], yp[:, :512])
                    nc.sync.dma_start(
                        yT_d[128 * ot:128 * (ot + 1), 512 * qc:512 * (qc + 1)],
                        yo[:])

            import os
            _order = os.environ.get("EMIT_ORDER", "A")
            def load_wpT():
                for i in range(2):
                    [nc.sync, nc.scalar][i % 2].dma_start(wpT[i][:], wpT_d[128 * i:128 * (i + 1), :])
            if _order == "A":
                for t in (0, 2, 1, 3):
                    norm_tile(t)
                load_wpT()
                for qc in range(QC):
                    for hl in range(HPC):
                        attn_chain(qc, hl)
                    proj_qc(qc)
            elif _order == "B":
                norm_tile(0); norm_tile(2)
                load_wpT()
                attn_chain(0, 0)
                norm_tile(1); norm_tile(3)
                attn_chain(0, 1); attn_chain(0, 2); attn_chain(0, 3)
                proj_qc(0)
                for qc in range(1, QC):
                    for hl in range(HPC):
                        attn_chain(qc, hl)
                    proj_qc(qc)
            elif _order == "C":
                norm_tile(0); norm_tile(2)
                load_wpT()
                attn_chain(0, 0); attn_chain(0, 1)
                norm_tile(1); norm_tile(3)
                attn_chain(1, 0); attn_chain(1, 1)
                attn_chain(0, 2); attn_chain(0, 3)
                proj_qc(0)
                attn_chain(1, 2); attn_chain(1, 3)
                proj_qc(1)
                for qc in range(2, QC):
                    for hl in range(HPC):
                        attn_chain(qc, hl)
                    proj_qc(qc)

    nc.compile()
    return nc


# ---------------- host-side data prep ----------------

def rope_tables():
    inv_freq = 1.0 / (ROPE_THETA ** (np.arange(0, D, 2, dtype=np.float32) / D))  # [32]
    cos = np.ones((32, N), np.float32)
    sin = np.zeros((32, N), np.float32)
    start = 0
    for seg in ROPE_SEGMENTS:
        ang = np.arange(seg, dtype=np.float32)[None, :] * inv_freq[:, None]  # [32, seg]
        cos[:, start:start + seg] = np.cos(ang)
        sin[:, start:start + seg] = np.sin(ang)
        start += seg
    cosF = np.empty((128, N), np.float32)
    sinF = np.empty((128, N), np.float32)
    for hp in range(2):
        r = 64 * hp
        cosF[r:r + 32] = cos; cosF[r + 32:r + 64] = cos
        sinF[r:r + 32] = -sin; sinF[r + 32:r + 64] = sin
    return cosF, sinF


def core_inputs(core, x, qkv_w, qkv_b, qn_w, kn_w, proj_w):
    b, g = divmod(core, 4)
    heads = [4 * g + i for i in range(HPC)]
    xT = np.ascontiguousarray(x[b].T)  # [C, N]
    q_rows = np.concatenate([np.arange(64 * h, 64 * h + 64) for h in heads])
    k_rows = q_rows + C
    v_rows = q_rows + 2 * C
    qk_rows = np.concatenate([q_rows, k_rows])
    wqkT = np.ascontiguousarray(qkv_w[qk_rows].T)        # [C, 512]
    bqk = np.ascontiguousarray(qkv_b[qk_rows].reshape(4, 128).T)  # [128, 4]
    wvT = np.zeros((C, 260), np.float32)
    bv = np.zeros((260,), np.float32)
    for hl in range(HPC):
        wvT[:, 65 * hl:65 * hl + 64] = qkv_w[v_rows[64 * hl:64 * hl + 64]].T
        bv[65 * hl:65 * hl + 64] = qkv_b[v_rows[64 * hl:64 * hl + 64]]
        bv[65 * hl + 64] = 1.0
    bv128 = np.broadcast_to(bv, (128, 260)).copy()
    cosF, sinF = rope_tables()
    wq = np.tile(qn_w.astype(np.float32), 2)[:, None].copy()  # [128,1]
    wk = np.tile(kn_w.astype(np.float32), 2)[:, None].copy()
    ind = np.zeros((128, 33), np.float32)
    ind[0:64, 0] = 1.0; ind[64:128, 32] = 1.0
    wpT = np.ascontiguousarray(proj_w[:, 256 * g:256 * (g + 1)].T)  # [256, C]
    return {
        "xT": xT, "wqkT": wqkT, "bqk": bqk, "wvT": wvT, "bv": bv128,
        "cosF": cosF, "sinF": sinF, "wq": wq, "wk": wk, "ind": ind, "wpT": wpT,
    }


def gather(results, proj_b):
    y = np.empty((B, N, C), np.float32)
    for b in range(B):
        acc = np.zeros((C, N), np.float32)
        for g in range(4):
            acc += results[4 * b + g]["yT"]
        y[b] = acc.T + proj_b[None, :]
    return y


class Runner:
    """Compiled SPMD runner (jit once, execute many) mirroring run_bass_via_pjrt."""

    def __init__(self, nc, n_cores=8):
        import jax
        import numpy as _np
        from jax.sharding import Mesh, PartitionSpec
        from jax.experimental.shard_map import shard_map
        import concourse.mybir as _mybir
        from concourse import bass2jax
        from concourse.bass2jax import _bass_exec_p, install_neuronx_cc_hook, partition_id_tensor

        install_neuronx_cc_hook()
        self.n_cores = n_cores
        partition_name = nc.partition_id_tensor.name if nc.partition_id_tensor else None
        in_names, out_names, out_avals, zero_outs = [], [], [], []
        for alloc in nc.m.functions[0].allocations:
            if not isinstance(alloc, _mybir.MemoryLocationSet):
                continue
            name = alloc.memorylocations[0].name
            if alloc.kind == "ExternalInput":
                if name != partition_name:
                    in_names.append(name)
            elif alloc.kind == "ExternalOutput":
                out_names.append(name)
                shape = tuple(alloc.tensor_shape)
                dtype = _mybir.dt.np(alloc.dtype)
                out_avals.append(jax.core.ShapedArray(shape, dtype))
                zero_outs.append(_np.zeros(shape, dtype))
        self.in_names, self.out_names = in_names, out_names
        self.out_avals, self.zero_outs = out_avals, zero_outs
        n_params, n_outs = len(in_names), len(out_avals)
        self.n_params = n_params
        all_in_names = list(in_names) + list(out_names)
        if partition_name is not None:
            all_in_names.append(partition_name)

        def _body(*args):
            operands = list(args)
            if partition_name is not None:
                operands.append(partition_id_tensor())
            outs = _bass_exec_p.bind(
                *operands,
                out_avals=tuple(out_avals),
                in_names=tuple(all_in_names),
                out_names=tuple(out_names),
                lowering_input_output_aliases=(),
                sim_require_finite=True,
                sim_require_nnan=True,
                nc=nc,
            )
            return tuple(outs)

        devices = jax.devices()[:n_cores]
        mesh = Mesh(_np.asarray(devices), ("core",))
        in_specs = (PartitionSpec("core"),) * (n_params + n_outs)
        out_specs = (PartitionSpec("core"),) * n_outs
        self._fn = jax.jit(
            shard_map(_body, mesh=mesh, in_specs=in_specs, out_specs=out_specs,
                      check_rep=False),
            keep_unused=True,
        )
        self._jax = jax

    def prep(self, in_maps):
        import numpy as _np
        per_core = [[_np.asarray(m[nm]) for nm in self.in_names] for m in in_maps]
        concat_in = [
            _np.concatenate([per_core[c][i] for c in range(self.n_cores)], axis=0)
            for i in range(self.n_params)
        ]
        concat_zeros = [
            _np.zeros((self.n_cores * z.shape[0], *z.shape[1:]), z.dtype)
            for z in self.zero_outs
        ]
        return concat_in + concat_zeros

    def run_device(self, dev_args):
        outs = self._fn(*dev_args)
        self._jax.block_until_ready(outs)
        return outs

    def run(self, in_maps):
        import numpy as _np
        outs = self.run_device(self.prep(in_maps))
        return [
            {nm: _np.asarray(outs[i]).reshape(self.n_cores, *self.out_avals[i].shape)[c]
             for i, nm in enumerate(self.out_names)}
            for c in range(self.n_cores)
        ]


def make_chained_fn(runner, nc, M):
    """Build a jitted fn executing the kernel M times serially (dep-chained)."""
    import jax
    import jax.numpy as jnp
    import numpy as _np
    from jax.sharding import Mesh, PartitionSpec
    from jax.experimental.shard_map import shard_map
    from concourse.bass2jax import _bass_exec_p, partition_id_tensor
    import concourse.mybir as _mybir

    partition_name = nc.partition_id_tensor.name if nc.partition_id_tensor else None
    all_in_names = list(runner.in_names) + list(runner.out_names)
    if partition_name is not None:
        all_in_names.append(partition_name)
    out_avals = runner.out_avals

    def _body(*args):
        n = runner.n_params
        ins = list(args[:n])
        zouts = list(args[n:])
        y = None
        for it in range(M):
            operands = list(ins)
            if y is not None:
                # fake dependency: perturb first input by 0*y[0,0]
                operands[0] = ins[0] + y[0].ravel()[0] * 0.0
            operands += zouts
            if partition_name is not None:
                operands.append(partition_id_tensor())
            y = _bass_exec_p.bind(
                *operands,
                out_avals=tuple(out_avals),
                in_names=tuple(all_in_names),
                out_names=tuple(runner.out_names),
                lowering_input_output_aliases=(),
                sim_require_finite=True,
                sim_require_nnan=True,
                nc=nc,
            )
        return tuple(y)

    devices = jax.devices()[:runner.n_cores]
    mesh = Mesh(_np.asarray(devices), ("core",))
    nio = runner.n_params + len(runner.out_names)
    return jax.jit(shard_map(_body, mesh=mesh,
                             in_specs=(PartitionSpec("core"),) * nio,
                             out_specs=(PartitionSpec("core"),) * len(runner.out_names),
                             check_rep=False), keep_unused=True)


_CACHE = {}


def _get_kernel(w_is_ones):
    key = bool(w_is_ones)
    if key not in _CACHE:
        nc = build_kernel(w_is_ones=key)
        _CACHE[key] = (nc, Runner(nc, 8))
    return _CACHE[key]


def kernel(x, qkv_w, qkv_b, qn_w, kn_w, proj_w, proj_b):
    x = np.ascontiguousarray(np.asarray(x, dtype=np.float32))
    qkv_w = np.ascontiguousarray(np.asarray(qkv_w, dtype=np.float32))
    qkv_b = np.ascontiguousarray(np.asarray(qkv_b, dtype=np.float32))
    qn_w = np.ascontiguousarray(np.asarray(qn_w, dtype=np.float32))
    kn_w = np.ascontiguousarray(np.asarray(kn_w, dtype=np.float32))
    proj_w = np.ascontiguousarray(np.asarray(proj_w, dtype=np.float32))
    proj_b = np.ascontiguousarray(np.asarray(proj_b, dtype=np.float32))
    w_is_ones = bool(np.all(qn_w == 1.0) and np.all(kn_w == 1.0))
    nc, runner = _get_kernel(w_is_ones)
    in_maps = [core_inputs(c, x, qkv_w, qkv_b, qn_w, kn_w, proj_w)
               for c in range(8)]
    results = runner.run(in_maps)
    return gather(results, proj_b)



# revision 3
# speedup vs baseline: 1.7497x; 1.7497x over previous
"""Trainium2 Bass kernel for nn_Attention_59030030516520.

Fused attention block: qkv projection + per-head RMSNorm + segmented RoPE +
softmax attention + output projection, distributed over 8 NeuronCores as
batch(2) x head-groups(4).  Each core computes 4 heads of one batch element
and a partial output projection; the host sums the partials and adds the bias.

Matmuls run in float32r (TF32-class, full PE rate); softmax exploits the bound
|q.k|/sqrt(D) <= sqrt(D) after RMSNorm so no max-subtraction pass is needed.
Scores are computed transposed (S^T = k q^T) so softmax rowsums come free via
a phantom all-ones v'-column and no transposes of the probability matrix are
required.
"""
import sys
sys.path.insert(0, "/opt/trn_rl_repo")
import numpy as np
import concourse.bass as bass
import concourse.mybir as mybir
import concourse.tile as tile
from concourse import bacc

F32 = mybir.dt.float32
F32R = mybir.dt.float32r
AF = mybir.ActivationFunctionType
ALU = mybir.AluOpType

B, N, C = 2, 2048, 1024
H, D = 16, 64
HPC = 4            # heads per core
NT = N // 128      # 16 seq tiles
QC = N // 512      # 4 q-chunks
EPS = 1e-6
SCALE = 1.0 / np.sqrt(D)
ROPE_SEGMENTS = (1024, 512)
NROPE = 1536
ROPE_THETA = 10000.0


def build_kernel(w_is_ones=True, repeat=1):
    nc = bacc.Bacc("TRN2", target_bir_lowering=False, debug=False)

    # ---- DRAM I/O (per-core) ----
    xT_d = nc.dram_tensor("xT", [C, N], F32R, kind="ExternalInput")           # x[b].T
    wqkT_d = nc.dram_tensor("wqkT", [C, 512], F32R, kind="ExternalInput")     # q,k weights.T (4 heads)
    bqk_d = nc.dram_tensor("bqk", [128, 4], F32, kind="ExternalInput")        # q,k bias per feature tile
    wvT_d = nc.dram_tensor("wvT", [C, 260], F32R, kind="ExternalInput")       # v weights.T + phantom cols
    bv_d = nc.dram_tensor("bv", [128, 260], F32, kind="ExternalInput")        # v bias row broadcast + ones at phantom
    cosF_d = nc.dram_tensor("cosF", [128, N], F32, kind="ExternalInput")
    sinF_d = nc.dram_tensor("sinF", [128, N], F32, kind="ExternalInput")
    wq_d = nc.dram_tensor("wq", [128, 1], F32, kind="ExternalInput")          # qn_w tiled
    wk_d = nc.dram_tensor("wk", [128, 1], F32, kind="ExternalInput")
    ind_d = nc.dram_tensor("ind", [128, 33], F32R, kind="ExternalInput")       # 64-row group indicator
    wpT_d = nc.dram_tensor("wpT", [256, C], F32R, kind="ExternalInput")       # proj weights slice.T
    yT_d = nc.dram_tensor("yT", [C, N], F32, kind="ExternalOutput")           # partial proj out.T

    with tile.TileContext(nc) as tc:
        with (
            tc.tile_pool(name="pers", bufs=1) as pers,     # persistent tensors (unique tags)
            tc.tile_pool(name="big", bufs=11) as big,      # recycled [128,2048] working tiles
            tc.tile_pool(name="vp", bufs=16) as vpool,     # v' tiles live through attention
            tc.tile_pool(name="sm", bufs=4) as sm,         # small working tiles
            tc.tile_pool(name="ps", bufs=2, space="PSUM") as psum_s,   # 2x2 banks
            tc.tile_pool(name="po", bufs=2, space="PSUM") as psum_o,   # 2 banks
            tc.tile_pool(name="pm", bufs=2, space="PSUM") as psum_m,   # 2 banks
        ):
          for _rep in range(repeat):
            # ---- load weights/constants ----
            wqkT = [pers.tile([128, 512], F32R, tag=f"wqk{i}", name=f"wqk{i}") for i in range(8)]
            nc.sync.dma_start(wqkT[0][:], wqkT_d[0:128, :])
            nc.scalar.dma_start(wqkT[1][:], wqkT_d[128:256, :])
            xT = [big.tile([128, N], F32R, tag="big", name=f"xT{i}") for i in range(8)]
            _eng = [nc.sync, nc.scalar]
            for i in range(8):
                _eng[i % 2].dma_start(xT[i][:], xT_d[128 * i:128 * (i + 1), :])

            for i in range(2, 8):
                [nc.sync, nc.scalar][i % 2].dma_start(wqkT[i][:], wqkT_d[128 * i:128 * (i + 1), :])
            wvT = [pers.tile([128, 260], F32R, tag=f"wv{i}", name=f"wv{i}") for i in range(8)]
            for i in range(8):
                [nc.sync, nc.scalar][i % 2].dma_start(wvT[i][:], wvT_d[128 * i:128 * (i + 1), :])
            wpT = [pers.tile([128, C], F32R, tag=f"wp{i}", name=f"wp{i}") for i in range(2)]
            bqk = pers.tile([128, 4], F32, tag="bqk")
            nc.sync.dma_start(bqk[:], bqk_d[:])
            bv = pers.tile([128, 260], F32, tag="bv")
            nc.scalar.dma_start(bv[:], bv_d[:])
            wq = pers.tile([128, 1], F32, tag="wq")
            nc.sync.dma_start(wq[:], wq_d[:])
            wk = pers.tile([128, 1], F32, tag="wk")
            nc.scalar.dma_start(wk[:], wk_d[:])
            ind = pers.tile([128, 33], F32R, tag="ind")
            nc.sync.dma_start(ind[:], ind_d[:])
            eps_t = pers.tile([64, 1], F32, tag="eps", name="eps_t")
            nc.vector.memset(eps_t[:], EPS)

            # ---- qkv: q,k channel-major [feature, seq] ----
            # qkf[0],qkf[1] = q heads (0,1),(2,3); qkf[2],qkf[3] = k heads
            # fp32 scratch lives in the persistent f32r tiles (bitcast views)
            qkf = [pers.tile([128, N], F32R, tag=f"qkf{t}", name=f"qkf{t}") for t in range(4)]
            raw = [big.tile([128, N], F32, tag="big", name=f"raw{t}") for t in range(4)]
            for ft in (0, 2, 1, 3):
                for half in range(2):
                    ps = psum_s.tile([128, 1024], F32, tag="s")
                    for ci in range(8):
                        for q2 in range(2):
                            qc = 2 * half + q2
                            nc.tensor.matmul(
                                ps[:, 512 * q2:512 * (q2 + 1)],
                                wqkT[ci][:, 128 * ft:128 * (ft + 1)],
                                xT[ci][:, 512 * qc:512 * (qc + 1)],
                                start=(ci == 0), stop=(ci == 7),
                            )
                    # r = psum + bias (per-partition)
                    nc.vector.tensor_scalar(raw[ft][:, 1024 * half:1024 * (half + 1)],
                                            ps[:], bqk[:, ft:ft + 1], None, ALU.add)

            # ---- v: seq-major [seq, 65*4] with phantom ones columns ----
            vp = []
            for st in range(NT):
                ps = psum_m.tile([128, 512], F32, tag="m")
                for ci in range(8):
                    nc.tensor.matmul(
                        ps[:, :260],
                        xT[ci][:, 128 * st:128 * (st + 1)],
                        wvT[ci][:],
                        start=(ci == 0), stop=(ci == 7),
                    )
                v = vpool.tile([128, 260], F32R, tag="v")
                nc.vector.tensor_tensor(v[:], ps[:, :260], bv[:], ALU.add)
                vp.append(v)

            aT = [pers.tile([128, N], F32R, tag=f"aT{i}", name=f"aT{i}") for i in range(2)]
            cosF = big.tile([128, N], F32, tag="big", name="cosF")
            nc.scalar.dma_start(cosF[:], cosF_d[:])
            sinF = big.tile([128, N], F32, tag="big", name="sinF")
            nc.scalar.dma_start(sinF[:], sinF_d[:])

            # ---- per-tile: RMSNorm stats + RoPE + ir scaling (t order 0,2 first
            # so attention heads 0/1 can start while tiles 1/3 normalize) ----
            def norm_tile(t):
                sq = big.tile([128, N], F32R, tag="big")
                ir = sm.tile([64, N], F32, tag="ir", bufs=1, name=f"ir{t}")
                for qc in range(QC):
                    nc.vector.tensor_tensor(sq[:, 512 * qc:512 * (qc + 1)],
                                            raw[t][:, 512 * qc:512 * (qc + 1)],
                                            raw[t][:, 512 * qc:512 * (qc + 1)], ALU.mult)
                    pr = psum_m.tile([128, 512], F32, tag="m")
                    sl = sq[:, 512 * qc:512 * (qc + 1)]
                    nc.tensor.matmul(pr[0:33, :512], ind[:], sl, start=True, stop=True)
                    # ir = 1/sqrt(ssq/D + eps); all norm Sqrts precede all
                    # softmax Exps (order A), so only 2 ACT table loads total
                    nc.scalar.activation(
                        ir[0:33, 512 * qc:512 * (qc + 1)],
                        pr[0:33, :512], AF.Sqrt,
                        bias=eps_t[0:33], scale=1.0 / D,
                    )
                    nc.vector.reciprocal(ir[0:33, 512 * qc:512 * (qc + 1)],
                                         ir[0:33, 512 * qc:512 * (qc + 1)])
                if not w_is_ones:
                    # exact general w: scale channels before rope (after stats)
                    wvec = wq if t < 2 else wk
                    nc.vector.tensor_scalar(raw[t][:], raw[t][:], wvec[:], None, ALU.mult)
                sw = big.tile([128, NROPE], F32, tag="big")
                for blk in range(4):
                    sfrom = (blk // 2) * 64 + (32 if blk % 2 == 0 else 0)
                    sto = (blk // 2) * 64 + (0 if blk % 2 == 0 else 32)
                    [nc.scalar, nc.sync][blk % 2].dma_start(sw[sto:sto + 32, :], raw[t][sfrom:sfrom + 32, 0:NROPE])
                # rope out-of-place so stats and rope chains overlap
                ropeo = big.tile([128, NROPE], F32, tag="big")
                nc.vector.tensor_tensor(ropeo[:], raw[t][:, 0:NROPE], cosF[:, 0:NROPE], ALU.mult)
                nc.vector.tensor_tensor(sw[:], sw[:], sinF[:, 0:NROPE], ALU.mult)
                nc.vector.tensor_tensor(ropeo[:], ropeo[:], sw[:], ALU.add)
                # broadcast ir rows to 64-row blocks (partition_broadcast only
                # works base0->base0 on HW; shift the second half with DMA)
                bc = big.tile([128, N], F32, tag="big")
                tmpb = big.tile([64, N], F32, tag="big")
                nc.gpsimd.dma_start(tmpb[0:1, :], ir[32:33, :])
                nc.gpsimd.partition_broadcast(bc[0:64, :], ir[0:1, :])
                nc.gpsimd.partition_broadcast(tmpb[0:64, :], tmpb[0:1, :])
                nc.sync.dma_start(bc[64:128, :], tmpb[0:64, :])
                nc.vector.tensor_tensor(qkf[t][:, 0:NROPE], bc[:, 0:NROPE], ropeo[:], ALU.mult)
                nc.vector.tensor_tensor(qkf[t][:, NROPE:N], bc[:, NROPE:N], raw[t][:, NROPE:N], ALU.mult)


            # ---- attention chain for one (qc, head) ----
            def attn_chain(qc, hl):
                ti, ro = hl // 2, 64 * (hl % 2)
                qf, kf = qkf[ti], qkf[2 + ti]
                po = psum_o.tile([128, 512], F32, tag="o", name=f"po{qc}_{hl}")
                for grp in range(8):
                    s2 = psum_s.tile([128, 1024], F32, tag="s", name=f"s{qc}_{hl}_{grp}")
                    for b2 in range(2):
                        t = 2 * grp + b2
                        nc.tensor.matmul(
                            s2[:, 512 * b2:512 * (b2 + 1)],
                            kf[ro:ro + 64, 128 * t:128 * (t + 1)],
                            qf[ro:ro + 64, 512 * qc:512 * (qc + 1)],
                            start=True, stop=True,
                        )
                    p2 = big.tile([128, 1024], F32R, tag="big", name=f"p{qc}_{hl}_{grp}")
                    nc.scalar.activation(p2[:], s2[:], AF.Exp, scale=float(SCALE))
                    for b2 in range(2):
                        t = 2 * grp + b2
                        nc.tensor.matmul(
                            po[0:65, :512],
                            vp[t][:, 65 * hl:65 * (hl + 1)],
                            p2[:, 512 * b2:512 * (b2 + 1)],
                            start=(t == 0), stop=(t == 15),
                        )
                # normalize: recip of rowsum (row 64), broadcast, multiply
                rs = sm.tile([128, 512], F32, tag="rs", bufs=1, name=f"rs{qc}_{hl}")
                nc.vector.reciprocal(rs[64:65, :], po[64:65, :512])
                nc.gpsimd.dma_start(rs[0:1, :], rs[64:65, :])
                rbc = sm.tile([64, 512], F32, tag="rbc", bufs=1, name=f"rbc{qc}_{hl}")
                nc.gpsimd.partition_broadcast(rbc[:], rs[0:1, :])
                if hl % 2 == 0:
                    nc.vector.tensor_tensor(
                        aT[ti][0:64, 512 * qc:512 * (qc + 1)],
                        po[0:64, :512], rbc[:], ALU.mult)
                else:
                    tmp = sm.tile([64, 512], F32R, tag="tmp", bufs=1, name=f"tmp{qc}_{hl}")
                    nc.vector.tensor_tensor(tmp[:], po[0:64, :512], rbc[:], ALU.mult)
                    nc.scalar.dma_start(aT[ti][64:128, 512 * qc:512 * (qc + 1)], tmp[:])

            def proj_qc(qc):
                for ot in range(8):
                    yp = psum_m.tile([128, 512], F32, tag="m", name=f"yp{qc}_{ot}")
                    for c2 in range(2):
                        nc.tensor.matmul(
                            yp[:, :512],
                            wpT[c2][:, 128 * ot:128 * (ot + 1)],
                            aT[c2][:, 512 * qc:512 * (qc + 1)],
                            start=(c2 == 0), stop=(c2 == 1),
                        )
                    yo = sm.tile([128, 512], F32, tag="yo", name=f"yo{qc}_{ot}", bufs=2)
                    if qc == QC - 1 and ot % 2 == 1:
                        nc.scalar.copy(yo[:], yp[:, :512])
                    else:
                        nc.vector.tensor_copy(yo[:], yp[:, :512])
                    nc.sync.dma_start(
                        yT_d[128 * ot:128 * (ot + 1), 512 * qc:512 * (qc + 1)],
                        yo[:])

            import os
            _order = os.environ.get("EMIT_ORDER", "A")
            def load_wpT():
                for i in range(2):
                    [nc.sync, nc.scalar][i % 2].dma_start(wpT[i][:], wpT_d[128 * i:128 * (i + 1), :])
            if _order == "A":
                for t in (0, 2, 1, 3):
                    norm_tile(t)
                load_wpT()
                for qc in range(QC):
                    for hl in range(HPC):
                        attn_chain(qc, hl)
                    proj_qc(qc)
            elif _order == "B":
                norm_tile(0); norm_tile(2)
                load_wpT()
                attn_chain(0, 0)
                norm_tile(1); norm_tile(3)
                attn_chain(0, 1); attn_chain(0, 2); attn_chain(0, 3)
                proj_qc(0)
                for qc in range(1, QC):
                    for hl in range(HPC):
                        attn_chain(qc, hl)
                    proj_qc(qc)
            elif _order == "C":
                norm_tile(0); norm_tile(2)
                load_wpT()
                attn_chain(0, 0); attn_chain(0, 1)
                norm_tile(1); norm_tile(3)
                attn_chain(1, 0); attn_chain(1, 1)
                attn_chain(0, 2); attn_chain(0, 3)
                proj_qc(0)
                attn_chain(1, 2); attn_chain(1, 3)
                proj_qc(1)
                for qc in range(2, QC):
                    for hl in range(HPC):
                        attn_chain(qc, hl)
                    proj_qc(qc)

    nc.compile()
    return nc


# ---------------- host-side data prep ----------------

def rope_tables():
    inv_freq = 1.0 / (ROPE_THETA ** (np.arange(0, D, 2, dtype=np.float32) / D))  # [32]
    cos = np.ones((32, N), np.float32)
    sin = np.zeros((32, N), np.float32)
    start = 0
    for seg in ROPE_SEGMENTS:
        ang = np.arange(seg, dtype=np.float32)[None, :] * inv_freq[:, None]  # [32, seg]
        cos[:, start:start + seg] = np.cos(ang)
        sin[:, start:start + seg] = np.sin(ang)
        start += seg
    cosF = np.empty((128, N), np.float32)
    sinF = np.empty((128, N), np.float32)
    for hp in range(2):
        r = 64 * hp
        cosF[r:r + 32] = cos; cosF[r + 32:r + 64] = cos
        sinF[r:r + 32] = -sin; sinF[r + 32:r + 64] = sin
    return cosF, sinF


def core_inputs(core, x, qkv_w, qkv_b, qn_w, kn_w, proj_w):
    b, g = divmod(core, 4)
    heads = [4 * g + i for i in range(HPC)]
    xT = np.ascontiguousarray(x[b].T)  # [C, N]
    q_rows = np.concatenate([np.arange(64 * h, 64 * h + 64) for h in heads])
    k_rows = q_rows + C
    v_rows = q_rows + 2 * C
    qk_rows = np.concatenate([q_rows, k_rows])
    wqkT = np.ascontiguousarray(qkv_w[qk_rows].T)        # [C, 512]
    bqk = np.ascontiguousarray(qkv_b[qk_rows].reshape(4, 128).T)  # [128, 4]
    wvT = np.zeros((C, 260), np.float32)
    bv = np.zeros((260,), np.float32)
    for hl in range(HPC):
        wvT[:, 65 * hl:65 * hl + 64] = qkv_w[v_rows[64 * hl:64 * hl + 64]].T
        bv[65 * hl:65 * hl + 64] = qkv_b[v_rows[64 * hl:64 * hl + 64]]
        bv[65 * hl + 64] = 1.0
    bv128 = np.broadcast_to(bv, (128, 260)).copy()
    cosF, sinF = rope_tables()
    wq = np.tile(qn_w.astype(np.float32), 2)[:, None].copy()  # [128,1]
    wk = np.tile(kn_w.astype(np.float32), 2)[:, None].copy()
    ind = np.zeros((128, 33), np.float32)
    ind[0:64, 0] = 1.0; ind[64:128, 32] = 1.0
    wpT = np.ascontiguousarray(proj_w[:, 256 * g:256 * (g + 1)].T)  # [256, C]
    return {
        "xT": xT, "wqkT": wqkT, "bqk": bqk, "wvT": wvT, "bv": bv128,
        "cosF": cosF, "sinF": sinF, "wq": wq, "wk": wk, "ind": ind, "wpT": wpT,
    }


def gather(results, proj_b):
    y = np.empty((B, N, C), np.float32)
    for b in range(B):
        acc = np.zeros((C, N), np.float32)
        for g in range(4):
            acc += results[4 * b + g]["yT"]
        y[b] = acc.T + proj_b[None, :]
    return y


class Runner:
    """Compiled SPMD runner (jit once, execute many) mirroring run_bass_via_pjrt."""

    def __init__(self, nc, n_cores=8):
        import jax
        import numpy as _np
        from jax.sharding import Mesh, PartitionSpec
        from jax.experimental.shard_map import shard_map
        import concourse.mybir as _mybir
        from concourse import bass2jax
        from concourse.bass2jax import _bass_exec_p, install_neuronx_cc_hook, partition_id_tensor

        install_neuronx_cc_hook()
        self.n_cores = n_cores
        partition_name = nc.partition_id_tensor.name if nc.partition_id_tensor else None
        in_names, out_names, out_avals, zero_outs = [], [], [], []
        for alloc in nc.m.functions[0].allocations:
            if not isinstance(alloc, _mybir.MemoryLocationSet):
                continue
            name = alloc.memorylocations[0].name
            if alloc.kind == "ExternalInput":
                if name != partition_name:
                    in_names.append(name)
            elif alloc.kind == "ExternalOutput":
                out_names.append(name)
                shape = tuple(alloc.tensor_shape)
                dtype = _mybir.dt.np(alloc.dtype)
                out_avals.append(jax.core.ShapedArray(shape, dtype))
                zero_outs.append(_np.zeros(shape, dtype))
        self.in_names, self.out_names = in_names, out_names
        self.out_avals, self.zero_outs = out_avals, zero_outs
        n_params, n_outs = len(in_names), len(out_avals)
        self.n_params = n_params
        all_in_names = list(in_names) + list(out_names)
        if partition_name is not None:
            all_in_names.append(partition_name)

        def _body(*args):
            operands = list(args)
            if partition_name is not None:
                operands.append(partition_id_tensor())
            outs = _bass_exec_p.bind(
                *operands,
                out_avals=tuple(out_avals),
                in_names=tuple(all_in_names),
                out_names=tuple(out_names),
                lowering_input_output_aliases=(),
                sim_require_finite=True,
                sim_require_nnan=True,
                nc=nc,
            )
            return tuple(outs)

        devices = jax.devices()[:n_cores]
        mesh = Mesh(_np.asarray(devices), ("core",))
        in_specs = (PartitionSpec("core"),) * (n_params + n_outs)
        out_specs = (PartitionSpec("core"),) * n_outs
        self._fn = jax.jit(
            shard_map(_body, mesh=mesh, in_specs=in_specs, out_specs=out_specs,
                      check_rep=False),
            keep_unused=True,
        )
        self._jax = jax

    def prep(self, in_maps):
        import numpy as _np
        per_core = [[_np.asarray(m[nm]) for nm in self.in_names] for m in in_maps]
        concat_in = [
            _np.concatenate([per_core[c][i] for c in range(self.n_cores)], axis=0)
            for i in range(self.n_params)
        ]
        concat_zeros = [
            _np.zeros((self.n_cores * z.shape[0], *z.shape[1:]), z.dtype)
            for z in self.zero_outs
        ]
        return concat_in + concat_zeros

    def run_device(self, dev_args):
        outs = self._fn(*dev_args)
        self._jax.block_until_ready(outs)
        return outs

    def run(self, in_maps):
        import numpy as _np
        outs = self.run_device(self.prep(in_maps))
        return [
            {nm: _np.asarray(outs[i]).reshape(self.n_cores, *self.out_avals[i].shape)[c]
             for i, nm in enumerate(self.out_names)}
            for c in range(self.n_cores)
        ]


def make_chained_fn(runner, nc, M):
    """Build a jitted fn executing the kernel M times serially (dep-chained)."""
    import jax
    import jax.numpy as jnp
    import numpy as _np
    from jax.sharding import Mesh, PartitionSpec
    from jax.experimental.shard_map import shard_map
    from concourse.bass2jax import _bass_exec_p, partition_id_tensor
    import concourse.mybir as _mybir

    partition_name = nc.partition_id_tensor.name if nc.partition_id_tensor else None
    all_in_names = list(runner.in_names) + list(runner.out_names)
    if partition_name is not None:
        all_in_names.append(partition_name)
    out_avals = runner.out_avals

    def _body(*args):
        n = runner.n_params
        ins = list(args[:n])
        zouts = list(args[n:])
        y = None
        for it in range(M):
            operands = list(ins)
            if y is not None:
                # fake dependency: perturb first input by 0*y[0,0]
                operands[0] = ins[0] + y[0].ravel()[0] * 0.0
            operands += zouts
            if partition_name is not None:
                operands.append(partition_id_tensor())
            y = _bass_exec_p.bind(
                *operands,
                out_avals=tuple(out_avals),
                in_names=tuple(all_in_names),
                out_names=tuple(runner.out_names),
                lowering_input_output_aliases=(),
                sim_require_finite=True,
                sim_require_nnan=True,
                nc=nc,
            )
        return tuple(y)

    devices = jax.devices()[:runner.n_cores]
    mesh = Mesh(_np.asarray(devices), ("core",))
    nio = runner.n_params + len(runner.out_names)
    return jax.jit(shard_map(_body, mesh=mesh,
                             in_specs=(PartitionSpec("core"),) * nio,
                             out_specs=(PartitionSpec("core"),) * len(runner.out_names),
                             check_rep=False), keep_unused=True)


_CACHE = {}


def _get_kernel(w_is_ones):
    key = bool(w_is_ones)
    if key not in _CACHE:
        nc = build_kernel(w_is_ones=key)
        _CACHE[key] = (nc, Runner(nc, 8))
    return _CACHE[key]


def kernel(x, qkv_w, qkv_b, qn_w, kn_w, proj_w, proj_b):
    x = np.ascontiguousarray(np.asarray(x, dtype=np.float32))
    qkv_w = np.ascontiguousarray(np.asarray(qkv_w, dtype=np.float32))
    qkv_b = np.ascontiguousarray(np.asarray(qkv_b, dtype=np.float32))
    qn_w = np.ascontiguousarray(np.asarray(qn_w, dtype=np.float32))
    kn_w = np.ascontiguousarray(np.asarray(kn_w, dtype=np.float32))
    proj_w = np.ascontiguousarray(np.asarray(proj_w, dtype=np.float32))
    proj_b = np.ascontiguousarray(np.asarray(proj_b, dtype=np.float32))
    w_is_ones = bool(np.all(qn_w == 1.0) and np.all(kn_w == 1.0))
    nc, runner = _get_kernel(w_is_ones)
    in_maps = [core_inputs(c, x, qkv_w, qkv_b, qn_w, kn_w, proj_w)
               for c in range(8)]
    results = runner.run(in_maps)
    return gather(results, proj_b)



# revision 47
# speedup vs baseline: 2.7082x; 1.5478x over previous
"""Trainium2 Bass kernel for nn_Attention_59030030516520.

Fused attention block: qkv projection + per-head RMSNorm + segmented RoPE +
softmax attention + output projection, distributed over 8 NeuronCores as
batch(2) x head-groups(4).  Each core computes 4 heads of one batch element
and a partial output projection; the host sums the partials and adds the bias.

Matmuls run in float32r (TF32-class, full PE rate); softmax exploits the bound
|q.k|/sqrt(D) <= sqrt(D) after RMSNorm so no max-subtraction pass is needed.
Scores are computed transposed (S^T = k q^T) so softmax rowsums come free via
a phantom all-ones v'-column and no transposes of the probability matrix are
required.

v2 over the original baseline:
- input DMA split into demand-ordered chunks over 4 HWDGE queues, and the
  qkv loop runs seq-half-outer so the PE never waits on the second half of x
- score matmuls for the two heads of a pair run concurrently in the PE array
  (K=64 row-tiling: head A in rows 0-63, head B in rows 64-127)
- RMSNorm squares run on GpSimd, reciprocals use the fast custom-DVE approx
- PSUM reorganised into 2x[128,1024] + 2x[128,512] + 2x[128,512] pools
"""
import sys
sys.path.insert(0, "/opt/trn_rl_repo")
import numpy as np
import concourse.bass as bass
import concourse.mybir as mybir
import concourse.tile as tile
from concourse import bacc

F32 = mybir.dt.float32
F32R = mybir.dt.float32r
BF16 = mybir.dt.bfloat16
AF = mybir.ActivationFunctionType
ALU = mybir.AluOpType

B, N, C = 2, 2048, 1024
H, D = 16, 64
HPC = 4            # heads per core
NT = N // 128      # 16 seq tiles
QC = N // 512      # 4 q-chunks
EPS = 1e-6
SCALE = 1.0 / np.sqrt(D)
ROPE_SEGMENTS = (1024, 512)
NROPE = 1536
ROPE_THETA = 10000.0


def build_kernel(w_is_ones=True, repeat=1):
    nc = bacc.Bacc("TRN2", target_bir_lowering=False, debug=False)

    # ---- DRAM I/O (per-core) ----
    xT_d = nc.dram_tensor("xT", [C, N], F32R, kind="ExternalInput")           # x[b].T
    wqkT_d = nc.dram_tensor("wqkT", [C, 512], F32R, kind="ExternalInput")     # q,k weights.T (4 heads)
    bqk_d = nc.dram_tensor("bqk", [128, 4], F32, kind="ExternalInput")        # q,k bias per feature tile
    wvT_d = nc.dram_tensor("wvT", [C, 260], F32R, kind="ExternalInput")       # v weights.T + phantom cols
    bv_d = nc.dram_tensor("bv", [128, 260], F32, kind="ExternalInput")        # v bias row broadcast + ones at phantom
    cosF_d = nc.dram_tensor("cosF", [128, N], F32, kind="ExternalInput")
    sinF_d = nc.dram_tensor("sinF", [128, N], F32, kind="ExternalInput")
    wq_d = nc.dram_tensor("wq", [128, 1], F32, kind="ExternalInput")          # qn_w tiled
    wk_d = nc.dram_tensor("wk", [128, 1], F32, kind="ExternalInput")
    ind_d = nc.dram_tensor("ind", [128, 33], F32R, kind="ExternalInput")       # 64-row group indicator
    cst_d = nc.dram_tensor("cst", [128, 256], F32R, kind="ExternalInput")      # ones row + bc selector
    wpT_d = nc.dram_tensor("wpT", [256, C], F32R, kind="ExternalInput")       # proj weights slice.T
    yT_d = nc.dram_tensor("yT", [C, N], F32, kind="ExternalOutput")           # partial proj out.T
    import os
    _dbg = os.environ.get("KDBG") == "1"
    if _dbg:
        raw0_d = nc.dram_tensor("raw0_dbg", [128, N], F32, kind="ExternalOutput")
        ropeo0_d = nc.dram_tensor("ropeo0_dbg", [128, NROPE], F32, kind="ExternalOutput")
        sw0_d = nc.dram_tensor("sw0_dbg", [128, NROPE], F32, kind="ExternalOutput")
        ir0_d = nc.dram_tensor("ir0_dbg", [2, N], F32, kind="ExternalOutput")
        qkf0_d = nc.dram_tensor("qkf0_dbg", [128, N], BF16, kind="ExternalOutput")
        vp0_d = nc.dram_tensor("vp0_dbg", [128, 260], F32, kind="ExternalOutput")

    with tile.TileContext(nc) as tc:
        with (
            tc.tile_pool(name="pers", bufs=1) as pers,     # persistent tensors (unique tags)
            tc.tile_pool(name="big", bufs=11) as big,      # recycled [128,2048] working tiles
            tc.tile_pool(name="vp", bufs=16) as vpool,     # v' tiles live through attention
            tc.tile_pool(name="sm", bufs=4) as sm,         # small working tiles
            tc.tile_pool(name="ps", bufs=2, space="PSUM") as psum_s,   # 2x2 banks
            tc.tile_pool(name="po", bufs=2, space="PSUM") as psum_o,   # 2 banks
            tc.tile_pool(name="pm", bufs=2, space="PSUM") as psum_m,   # 2 banks
        ):
          for _rep in range(repeat):
            # ---- load weights/constants (demand-ordered, 2 HW queues) ----
            Q2 = [nc.sync, nc.scalar]
            # tiny constants first: they unblock the qkv bias adds and stats
            bqk = pers.tile([128, 4], F32, tag="bqk")
            nc.sync.dma_start(bqk[:], bqk_d[:])
            bv = pers.tile([128, 260], F32R, tag="bv")
            nc.scalar.dma_start(bv[:], bv_d[:].bitcast(F32R))
            wq = pers.tile([128, 1], F32, tag="wq")
            nc.sync.dma_start(wq[:], wq_d[:])
            wk = pers.tile([128, 1], F32, tag="wk")
            nc.scalar.dma_start(wk[:], wk_d[:])
            ind = pers.tile([128, 33], F32R, tag="ind")
            nc.sync.dma_start(ind[:], ind_d[:])
            wqkT = [pers.tile([128, 512], F32R, tag=f"wqk{i}", name=f"wqk{i}") for i in range(8)]
            xT = [big.tile([128, N], F32R, tag="big", name=f"xT{i}") for i in range(8)]
            # first halves of x + qk weights, interleaved per contraction tile
            for i in range(8):
                Q2[i % 2].dma_start(wqkT[i][:], wqkT_d[128 * i:128 * (i + 1), :])
                Q2[(i + 1) % 2].dma_start(xT[i][:, 0:1024], xT_d[128 * i:128 * (i + 1), 0:1024])
            # second halves of x
            for i in range(8):
                Q2[i % 2].dma_start(xT[i][:, 1024:2048], xT_d[128 * i:128 * (i + 1), 1024:2048])
            # v weights
            wvT = [pers.tile([128, 260], F32R, tag=f"wv{i}", name=f"wv{i}") for i in range(8)]
            for i in range(8):
                Q2[i % 2].dma_start(wvT[i][:], wvT_d[128 * i:128 * (i + 1), :])
            wpT = [pers.tile([128, C], F32R, tag=f"wp{i}", name=f"wp{i}") for i in range(2)]
            # cst[:, 0:128]: row 0 all ones (K=1 bias-broadcast matmuls);
            # cst[:, 128:256]: rows {0,1} hold the 2x128 selector that maps
            # ir row 0 -> partitions 0-63 and ir row 1 -> partitions 64-127
            cst = pers.tile([128, 256], F32R, tag="cst", name="cst")
            nc.sync.dma_start(cst[:], cst_d[:])

            # ---- qkv: q,k channel-major [feature, seq] ----
            qkf = [pers.tile([128, N], BF16, tag=f"qkf{t}", name=f"qkf{t}") for t in range(4)]
            raw = [big.tile([128, N], F32, tag="big", name=f"raw{t}") for t in range(4)]

            def qkv_ft(ft, half):
                ps = psum_s.tile([128, 1024], F32, tag="s")
                for ci in range(8):
                    for q2 in range(2):
                        qc = 2 * half + q2
                        nc.tensor.matmul(
                            ps[:, 512 * q2:512 * (q2 + 1)],
                            wqkT[ci][:, 128 * ft:128 * (ft + 1)],
                            xT[ci][:, 512 * qc:512 * (qc + 1)],
                            start=(ci == 0), stop=(ci == 7),
                        )
                nc.vector.tensor_scalar(raw[ft][:, 1024 * half:1024 * (half + 1)],
                                        ps[:], bqk[:, ft:ft + 1], None, ALU.add)

            # ---- per-tile RMSNorm stats: sq -> group-sum matmul -> Sqrt ->
            # fast reciprocal.  ir rows {0,32} hold the two head groups. ----
            irs = {}

            def norm_stats(t):
                sq = big.tile([128, N], F32R, tag="big")
                ir = sm.tile([2, N], F32R, tag="ir", bufs=2, name=f"ir{t}")
                irs[t] = ir
                for qc in range(QC):
                    nc.vector.tensor_tensor(sq[:, 512 * qc:512 * (qc + 1)],
                                            raw[t][:, 512 * qc:512 * (qc + 1)],
                                            raw[t][:, 512 * qc:512 * (qc + 1)], ALU.mult)
                    pr = psum_m.tile([128, 512], F32, tag="m")
                    sl = sq[:, 512 * qc:512 * (qc + 1)]
                    nc.tensor.matmul(pr[0:2, :512], ind[:, 0:2], sl, start=True, stop=True)
                    # ir = 1/sqrt(ssq/D) = sqrt(D * (1/ssq)): fast-approx
                    # reciprocal first (ssq/D ~ 1 so the reference's eps=1e-6
                    # is 6 orders below it - dropping it is a <1e-6 rel
                    # change), then Sqrt on ACT writes f32r-rounded ir for
                    # the broadcast matmul
                    rcp = sm.tile([2, 512], F32, tag="rcp", bufs=2, name=f"rcp{t}_{qc}")
                    nc.scalar.copy(rcp[0:2, :], pr[0:2, :512])
                    nc.vector.reciprocal_approx_fast(
                        out=rcp[0:2, :], in_=rcp[0:2, :])
                    nc.scalar.activation(
                        ir[0:2, 512 * qc:512 * (qc + 1)],
                        rcp[0:2, :], AF.Sqrt, scale=float(D),
                    )
                if not w_is_ones:
                    # exact general w: scale channels before rope (after stats)
                    wvec = wq if t < 2 else wk
                    nc.vector.tensor_scalar(raw[t][:], raw[t][:], wvec[:], None, ALU.mult)

            # seq-half-outer for tiles 0,2,1 (their first passes only need the
            # first half of x, already in flight); tile 3 runs last with both
            # halves because its raw recycles an xT ring slot (the big-tile
            # ring holds 8 xT + 4 raw + transients in 11 slots, so the fourth
            # raw can only be written once the v-phase frees the xT reads)
            for ft, half in ((0, 0), (2, 0), (1, 0), (0, 1), (2, 1), (1, 1), (3, 0), (3, 1)):
                qkv_ft(ft, half)

            # ---- v: seq-major [seq, 65*4] with phantom ones columns; the
            # bias rides as a K=1 ones-row x bias-row accumulate and the copy
            # runs on the (here idle) scalar engine, keeping the DVE out of
            # the v-phase entirely (its FIFO is owned by the qkv bias chain
            # whose tail waits for the v-phase to free xT ring slots) ----
            vp = []
            for st in range(NT):
                ps = psum_m.tile([128, 512], F32, tag="m")
                for ci in range(8):
                    nc.tensor.matmul(
                        ps[:, :260],
                        xT[ci][:, 128 * st:128 * (st + 1)],
                        wvT[ci][:],
                        start=(ci == 0), stop=False,
                    )
                nc.tensor.matmul(
                    ps[:, :260],
                    cst[0:1, 0:128],
                    bv[0:1, 0:260],
                    start=False, stop=True,
                )
                v = vpool.tile([128, 260], F32R, tag="v")
                nc.scalar.copy(v[:], ps[:, :260])
                vp.append(v)

            aT = [pers.tile([128, N], F32R, tag=f"aT{i}", name=f"aT{i}") for i in range(2)]
            # rope tables (only the roped 1536 columns are ever read); these
            # tiles recycle xT ring slots, so they allocate after the v-phase
            cosF = big.tile([128, N], F32, tag="big", name="cosF")
            nc.sync.dma_start(cosF[:, 0:NROPE], cosF_d[:, 0:NROPE])
            sinF = big.tile([128, N], F32, tag="big", name="sinF")
            nc.scalar.dma_start(sinF[:, 0:NROPE], sinF_d[:, 0:NROPE])

            # ---- per-tile RoPE + ir broadcast-scale into the final qkf ----
            def norm_apply(t):
                ir = irs[t]
                sw = big.tile([128, NROPE], F32, tag="big")
                for blk in range(4):
                    sfrom = (blk // 2) * 64 + (32 if blk % 2 == 0 else 0)
                    sto = (blk // 2) * 64 + (0 if blk % 2 == 0 else 32)
                    [nc.scalar, nc.sync][blk % 2].dma_start(sw[sto:sto + 32, :], raw[t][sfrom:sfrom + 32, 0:NROPE])
                # rope out-of-place so tiles pipeline
                ropeo = big.tile([128, NROPE], F32, tag="big")
                nc.vector.tensor_tensor(ropeo[:], raw[t][:, 0:NROPE], cosF[:, 0:NROPE], ALU.mult)
                nc.vector.tensor_tensor(sw[:], sw[:], sinF[:, 0:NROPE], ALU.mult)
                nc.vector.tensor_tensor(ropeo[:], ropeo[:], sw[:], ALU.add)
                if _dbg and t == 0:
                    nc.sync.dma_start(ropeo0_d[:], ropeo[:])
                    nc.sync.dma_start(sw0_d[:], sw[:])
                # broadcast ir rows {0,1} to the two 64-partition blocks in a
                # single K=2 selector matmul straight into PSUM; the per-chunk
                # normalize multiply then reads the PSUM tile
                for qc in range(QC):
                    bc = psum_m.tile([128, 512], F32, tag="m", name=f"bc{t}_{qc}")
                    nc.tensor.matmul(bc[:, :512], cst[0:2, 128:256],
                                     ir[0:2, 512 * qc:512 * (qc + 1)],
                                     start=True, stop=True)
                    src = ropeo[:, 512 * qc:512 * (qc + 1)] if qc < 3 else raw[t][:, NROPE:N]
                    nc.vector.tensor_tensor(qkf[t][:, 512 * qc:512 * (qc + 1)],
                                            bc[:], src, ALU.mult)

            norm_stats(0)
            norm_stats(2)
            norm_apply(0)
            norm_stats(1)
            norm_apply(2)
            norm_stats(3)
            norm_apply(1)
            norm_apply(3)
            if _dbg:
                nc.sync.dma_start(raw0_d[:], raw[0][:])
                nc.sync.dma_start(ir0_d[:], irs[0][:].bitcast(F32))
                nc.sync.dma_start(qkf0_d[:], qkf[0][:])
                nc.sync.dma_start(vp0_d[:], vp[0][:].bitcast(F32))

            def load_wpT():
                for i in range(2):
                    [nc.sync, nc.scalar][i % 2].dma_start(wpT[i][:], wpT_d[128 * i:128 * (i + 1), :])

            # ---- attention for one (qc, head-pair): the two heads' score
            # matmuls are K=64 row-tiles (A rows 0-63, B rows 64-127) and run
            # concurrently in the PE array ----
            def attn_pair(qc, ti):
                qf, kf = qkf[ti], qkf[2 + ti]
                hlA, hlB = 2 * ti, 2 * ti + 1
                poA = psum_o.tile([128, 512], F32, tag="o", name=f"poA{qc}_{ti}")
                poB = psum_o.tile([128, 512], F32, tag="o", name=f"poB{qc}_{ti}")
                for grp in range(8):
                    sA = psum_s.tile([128, 1024], F32, tag="s", name=f"sA{qc}_{ti}_{grp}")
                    sB = psum_s.tile([128, 1024], F32, tag="s", name=f"sB{qc}_{ti}_{grp}")
                    for b2 in range(2):
                        t = 2 * grp + b2
                        nc.tensor.matmul(
                            sA[:, 512 * b2:512 * (b2 + 1)],
                            kf[0:64, 128 * t:128 * (t + 1)],
                            qf[0:64, 512 * qc:512 * (qc + 1)],
                            start=True, stop=True,
                        )
                        nc.tensor.matmul(
                            sB[:, 512 * b2:512 * (b2 + 1)],
                            kf[64:128, 128 * t:128 * (t + 1)],
                            qf[64:128, 512 * qc:512 * (qc + 1)],
                            start=True, stop=True,
                        )
                    pA = big.tile([128, 1024], F32R, tag="big", name=f"pA{qc}_{ti}_{grp}")
                    nc.scalar.activation(pA[:], sA[:], AF.Exp, scale=float(SCALE))
                    pB = big.tile([128, 1024], F32R, tag="big", name=f"pB{qc}_{ti}_{grp}")
                    nc.scalar.activation(pB[:], sB[:], AF.Exp, scale=float(SCALE))
                    for b2 in range(2):
                        t = 2 * grp + b2
                        nc.tensor.matmul(
                            poA[0:65, :512],
                            vp[t][:, 65 * hlA:65 * (hlA + 1)],
                            pA[:, 512 * b2:512 * (b2 + 1)],
                            start=(t == 0), stop=(t == 15),
                        )
                        nc.tensor.matmul(
                            poB[0:65, :512],
                            vp[t][:, 65 * hlB:65 * (hlB + 1)],
                            pB[:, 512 * b2:512 * (b2 + 1)],
                            start=(t == 0), stop=(t == 15),
                        )
                # normalize: recip of rowsum (row 64), broadcast, multiply
                for hl, po in ((hlA, poA), (hlB, poB)):
                    rs = sm.tile([128, 512], F32, tag="rs", bufs=1, name=f"rs{qc}_{hl}")
                    nc.vector.reciprocal(rs[64:65, :], po[64:65, :512])
                    nc.sync.dma_start(rs[0:1, :], rs[64:65, :])
                    rbc = sm.tile([64, 512], F32, tag="rbc", bufs=1, name=f"rbc{qc}_{hl}")
                    nc.gpsimd.partition_broadcast(rbc[:], rs[0:1, :])
                    if hl % 2 == 0:
                        nc.vector.tensor_tensor(
                            aT[ti][0:64, 512 * qc:512 * (qc + 1)],
                            po[0:64, :512], rbc[:], ALU.mult)
                    else:
                        tmp = sm.tile([64, 512], F32R, tag="tmp", bufs=1, name=f"tmp{qc}_{hl}")
                        nc.vector.tensor_tensor(tmp[:], po[0:64, :512], rbc[:], ALU.mult)
                        nc.scalar.dma_start(aT[ti][64:128, 512 * qc:512 * (qc + 1)], tmp[:])

            def proj_qc(qc):
                for ot in range(8):
                    yp = psum_m.tile([128, 512], F32, tag="m", name=f"yp{qc}_{ot}")
                    for c2 in range(2):
                        nc.tensor.matmul(
                            yp[:, :512],
                            wpT[c2][:, 128 * ot:128 * (ot + 1)],
                            aT[c2][:, 512 * qc:512 * (qc + 1)],
                            start=(c2 == 0), stop=(c2 == 1),
                        )
                    yo = sm.tile([128, 512], F32, tag="yo", name=f"yo{qc}_{ot}", bufs=2)
                    if qc == QC - 1 and ot % 2 == 1:
                        nc.scalar.copy(yo[:], yp[:, :512])
                    else:
                        nc.vector.tensor_copy(yo[:], yp[:, :512])
                    nc.sync.dma_start(
                        yT_d[128 * ot:128 * (ot + 1), 512 * qc:512 * (qc + 1)],
                        yo[:])

            load_wpT()
            for qc in range(QC):
                for ti in range(2):
                    attn_pair(qc, ti)
                proj_qc(qc)

    nc.compile()
    return nc


# ---------------- host-side data prep ----------------

def rope_tables():
    inv_freq = 1.0 / (ROPE_THETA ** (np.arange(0, D, 2, dtype=np.float32) / D))  # [32]
    cos = np.ones((32, N), np.float32)
    sin = np.zeros((32, N), np.float32)
    start = 0
    for seg in ROPE_SEGMENTS:
        ang = np.arange(seg, dtype=np.float32)[None, :] * inv_freq[:, None]  # [32, seg]
        cos[:, start:start + seg] = np.cos(ang)
        sin[:, start:start + seg] = np.sin(ang)
        start += seg
    cosF = np.empty((128, N), np.float32)
    sinF = np.empty((128, N), np.float32)
    for hp in range(2):
        r = 64 * hp
        cosF[r:r + 32] = cos; cosF[r + 32:r + 64] = cos
        sinF[r:r + 32] = -sin; sinF[r + 32:r + 64] = sin
    return cosF, sinF


def core_inputs(core, x, qkv_w, qkv_b, qn_w, kn_w, proj_w):
    b, g = divmod(core, 4)
    heads = [4 * g + i for i in range(HPC)]
    xT = np.ascontiguousarray(x[b].T)  # [C, N]
    q_rows = np.concatenate([np.arange(64 * h, 64 * h + 64) for h in heads])
    k_rows = q_rows + C
    v_rows = q_rows + 2 * C
    qk_rows = np.concatenate([q_rows, k_rows])
    wqkT = np.ascontiguousarray(qkv_w[qk_rows].T)        # [C, 512]
    bqk = np.ascontiguousarray(qkv_b[qk_rows].reshape(4, 128).T)  # [128, 4]
    wvT = np.zeros((C, 260), np.float32)
    bv = np.zeros((260,), np.float32)
    for hl in range(HPC):
        wvT[:, 65 * hl:65 * hl + 64] = qkv_w[v_rows[64 * hl:64 * hl + 64]].T
        bv[65 * hl:65 * hl + 64] = qkv_b[v_rows[64 * hl:64 * hl + 64]]
        bv[65 * hl + 64] = 1.0
    bv128 = np.broadcast_to(bv, (128, 260)).copy()
    cosF, sinF = rope_tables()
    wq = np.tile(qn_w.astype(np.float32), 2)[:, None].copy()  # [128,1]
    wk = np.tile(kn_w.astype(np.float32), 2)[:, None].copy()
    ind = np.zeros((128, 33), np.float32)
    ind[0:64, 0] = 1.0; ind[64:128, 1] = 1.0
    cst = np.zeros((128, 256), np.float32)
    cst[0, 0:128] = 1.0                  # ones row for K=1 bias broadcasts
    cst[0, 128:192] = 1.0                # selector: ir row 0 -> parts 0-63
    cst[1, 192:256] = 1.0                # selector: ir row 1 -> parts 64-127
    wpT = np.ascontiguousarray(proj_w[:, 256 * g:256 * (g + 1)].T)  # [256, C]
    return {
        "xT": xT, "wqkT": wqkT, "bqk": bqk, "wvT": wvT, "bv": bv128,
        "cosF": cosF, "sinF": sinF, "wq": wq, "wk": wk, "ind": ind,
        "cst": cst, "wpT": wpT,
    }


def gather(results, proj_b):
    y = np.empty((B, N, C), np.float32)
    for b in range(B):
        acc = np.zeros((C, N), np.float32)
        for g in range(4):
            acc += results[4 * b + g]["yT"]
        y[b] = acc.T + proj_b[None, :]
    return y


class Runner:
    """Compiled SPMD runner (jit once, execute many) mirroring run_bass_via_pjrt."""

    def __init__(self, nc, n_cores=8):
        import jax
        import numpy as _np
        from jax.sharding import Mesh, PartitionSpec
        from jax.experimental.shard_map import shard_map
        import concourse.mybir as _mybir
        from concourse import bass2jax
        from concourse.bass2jax import _bass_exec_p, install_neuronx_cc_hook, partition_id_tensor

        install_neuronx_cc_hook()
        self.n_cores = n_cores
        partition_name = nc.partition_id_tensor.name if nc.partition_id_tensor else None
        in_names, out_names, out_avals, zero_outs = [], [], [], []
        for alloc in nc.m.functions[0].allocations:
            if not isinstance(alloc, _mybir.MemoryLocationSet):
                continue
            name = alloc.memorylocations[0].name
            if alloc.kind == "ExternalInput":
                if name != partition_name:
                    in_names.append(name)
            elif alloc.kind == "ExternalOutput":
                out_names.append(name)
                shape = tuple(alloc.tensor_shape)
                dtype = _mybir.dt.np(alloc.dtype)
                out_avals.append(jax.core.ShapedArray(shape, dtype))
                zero_outs.append(_np.zeros(shape, dtype))
        self.in_names, self.out_names = in_names, out_names
        self.out_avals, self.zero_outs = out_avals, zero_outs
        n_params, n_outs = len(in_names), len(out_avals)
        self.n_params = n_params
        all_in_names = list(in_names) + list(out_names)
        if partition_name is not None:
            all_in_names.append(partition_name)

        def _body(*args):
            operands = list(args)
            if partition_name is not None:
                operands.append(partition_id_tensor())
            outs = _bass_exec_p.bind(
                *operands,
                out_avals=tuple(out_avals),
                in_names=tuple(all_in_names),
                out_names=tuple(out_names),
                lowering_input_output_aliases=(),
                sim_require_finite=True,
                sim_require_nnan=True,
                nc=nc,
            )
            return tuple(outs)

        devices = jax.devices()[:n_cores]
        mesh = Mesh(_np.asarray(devices), ("core",))
        in_specs = (PartitionSpec("core"),) * (n_params + n_outs)
        out_specs = (PartitionSpec("core"),) * n_outs
        self._fn = jax.jit(
            shard_map(_body, mesh=mesh, in_specs=in_specs, out_specs=out_specs,
                      check_rep=False),
            keep_unused=True,
        )
        self._jax = jax

    def prep(self, in_maps):
        import numpy as _np
        per_core = [[_np.asarray(m[nm]) for nm in self.in_names] for m in in_maps]
        concat_in = [
            _np.concatenate([per_core[c][i] for c in range(self.n_cores)], axis=0)
            for i in range(self.n_params)
        ]
        concat_zeros = [
            _np.zeros((self.n_cores * z.shape[0], *z.shape[1:]), z.dtype)
            for z in self.zero_outs
        ]
        return concat_in + concat_zeros

    def run_device(self, dev_args):
        outs = self._fn(*dev_args)
        self._jax.block_until_ready(outs)
        return outs

    def run(self, in_maps):
        import numpy as _np
        outs = self.run_device(self.prep(in_maps))
        return [
            {nm: _np.asarray(outs[i]).reshape(self.n_cores, *self.out_avals[i].shape)[c]
             for i, nm in enumerate(self.out_names)}
            for c in range(self.n_cores)
        ]


_CACHE = {}


def _get_kernel(w_is_ones):
    key = bool(w_is_ones)
    if key not in _CACHE:
        nc = build_kernel(w_is_ones=key)
        _CACHE[key] = (nc, Runner(nc, 8))
    return _CACHE[key]


def kernel(x, qkv_w, qkv_b, qn_w, kn_w, proj_w, proj_b):
    x = np.ascontiguousarray(np.asarray(x, dtype=np.float32))
    qkv_w = np.ascontiguousarray(np.asarray(qkv_w, dtype=np.float32))
    qkv_b = np.ascontiguousarray(np.asarray(qkv_b, dtype=np.float32))
    qn_w = np.ascontiguousarray(np.asarray(qn_w, dtype=np.float32))
    kn_w = np.ascontiguousarray(np.asarray(kn_w, dtype=np.float32))
    proj_w = np.ascontiguousarray(np.asarray(proj_w, dtype=np.float32))
    proj_b = np.ascontiguousarray(np.asarray(proj_b, dtype=np.float32))
    w_is_ones = bool(np.all(qn_w == 1.0) and np.all(kn_w == 1.0))
    nc, runner = _get_kernel(w_is_ones)
    in_maps = [core_inputs(c, x, qkv_w, qkv_b, qn_w, kn_w, proj_w)
               for c in range(8)]
    results = runner.run(in_maps)
    return gather(results, proj_b)
